# revision 1
# baseline (speedup 1.0000x reference)
# Multi-head attention block (QKV proj + per-head q/k layernorm + softmax
# attention + output proj) on 8 Trainium2 NeuronCores.
#
# Sharding: data-parallel over (batch, query-half). Core c handles batch
# c//2, query tokens [ (c%2)*1024, (c%2+1)*1024 ). Each core computes K/V
# for its batch's full 2048 tokens; no cross-core communication, the host
# concatenates the 8 disjoint output chunks.
#
# On-device dataflow per core:
#   q,k are produced directly in [feature, token] layout (stationary =
#   Wqkv block, moving = xT) so no DMA transposes are needed; the qkv bias
#   rides the PSUM eviction as a per-partition ACT bias. LayerNorm stats
#   (per-head mean/rstd over D=64 = partition groups) come from ones-block
#   matmuls on the PE (ones scaled by 1/64 so PSUM holds the means); mu and
#   rstd are broadcast back over partitions with a DRAM-bounce DMA and
#   applied with two vector ops. q is stored zero-padded per head
#   ([q_head;0] / [0;q_head]) so score matmuls contract over K=128 and four
#   moving streams share one stationary load. v is natural-layout with a
#   ones column so the softmax normalizer Z rides attn@v as PSUM row 64.
#   The v bias is folded into beff = bv @ Wproj + bproj on the host
#   (softmax rows sum to one, so this is exact).
import contextlib

import numpy as np
import ml_dtypes

B, T, E = 4, 2048, 1024
H, D = 16, 64
P = 128
EPS = 1e-5
SCALE = 0.125  # 1/sqrt(D)
TQ = T // 2          # query tokens per core
KB = E // P          # contraction blocks
FC = E // P          # feature chunks for q/k (2 heads each)
MKV = T // P         # kv token tiles
NCORES = 8

_BUILT = {}
_last_in_maps = None
DEBUG = False


def _build_real(affine: bool):
    import concourse.bass as bass
    import concourse.bacc as bacc
    import concourse.tile as tile
    from concourse import mybir

    f32 = mybir.dt.float32
    bf16 = mybir.dt.bfloat16
    AF = mybir.ActivationFunctionType
    OP = mybir.AluOpType

    nc = bacc.Bacc("TRN2", target_bir_lowering=False)
    xT_q = nc.declare_dram_parameter("xT_q", [E, TQ], bf16, isOutput=False)
    xT_kv = nc.declare_dram_parameter("xT_kv", [E, T], bf16, isOutput=False)
    Wqkv = nc.declare_dram_parameter("Wqkv", [E, 3 * E], bf16, isOutput=False)
    bqkv = nc.declare_dram_parameter("bqkv", [3 * E], f32, isOutput=False)
    beff = nc.declare_dram_parameter("beff", [E], bf16, isOutput=False)
    if affine:
        q_gamma = nc.declare_dram_parameter("q_gamma", [D], f32, isOutput=False)
        q_beta = nc.declare_dram_parameter("q_beta", [D], f32, isOutput=False)
        k_gamma = nc.declare_dram_parameter("k_gamma", [D], f32, isOutput=False)
        k_beta = nc.declare_dram_parameter("k_beta", [D], f32, isOutput=False)
    Wproj = nc.declare_dram_parameter("Wproj", [E, E], bf16, isOutput=False)
    out = nc.declare_dram_parameter("out", [TQ, E], f32, isOutput=True)
    if DEBUG:
        dbg_q = nc.declare_dram_parameter("dbg_q", [P, 2, FC, TQ], bf16, isOutput=True)
        dbg_k = nc.declare_dram_parameter("dbg_k", [P, FC, T], bf16, isOutput=True)
        dbg_va = nc.declare_dram_parameter("dbg_va", [P, MKV, H, D + 1], bf16, isOutput=True)
        dbg_y = nc.declare_dram_parameter("dbg_y", [P, FC, TQ], bf16, isOutput=True)

    def bc_read(dst, tensor_ap, elem_off, reps, inner_ap):
        # broadcast-read: dst[p, ...] = src[elem_off + inner] for all p
        ap = bass.AP(tensor=tensor_ap.tensor,
                     offset=tensor_ap.offset + elem_off,
                     ap=[[0, reps], *inner_ap])
        nc.gpsimd.dma_start(out=dst, in_=ap)

    with tile.TileContext(nc) as tc, contextlib.ExitStack() as top:
        const = top.enter_context(tc.tile_pool(name="const", bufs=1))
        persist = top.enter_context(tc.tile_pool(name="persist", bufs=1))
        dr = top.enter_context(tc.tile_pool(name="dr", bufs=1, space="DRAM"))

        ones1 = const.tile([1, P], bf16)
        nc.vector.memset(ones1[:], 1.0)
        ones_bd = const.tile([P, 2], bf16)
        nc.vector.memset(ones_bd[:], 0.0)
        nc.vector.memset(ones_bd[0:64, 0:1], 1.0 / 64.0)
        nc.vector.memset(ones_bd[64:128, 1:2], 1.0 / 64.0)
        eps2 = const.tile([2, 1], f32)
        nc.vector.memset(eps2[:], EPS)
        bcol = const.tile([P, 16], f32)   # q/k bias, per-partition columns
        nc.sync.dma_start(out=bcol[:],
                          in_=bqkv[0:2 * E].rearrange("(c p) -> p c", p=P))
        beff_row = const.tile([1, E], bf16)
        nc.sync.dma_start(out=beff_row[:], in_=beff[:])
        if affine:
            gq_c = const.tile([P, 1], f32)
            bq_c = const.tile([P, 1], f32)
            gk_c = const.tile([P, 1], f32)
            bk_c = const.tile([P, 1], f32)
            for cc, src in ((gq_c, q_gamma), (bq_c, q_beta),
                            (gk_c, k_gamma), (bk_c, k_beta)):
                nc.sync.dma_start(out=cc[0:64, :], in_=src[:])
                nc.sync.dma_start(out=cc[64:128, :], in_=src[:])

        # qhat[:, 0] = [q_even; 0], qhat[:, 1] = [0; q_odd] (K=128 scores)
        qhat = persist.tile([P, 2, FC, TQ], bf16)
        nc.vector.memset(qhat[64:128, 0, :, :], 0.0)
        nc.vector.memset(qhat[0:64, 1, :, :], 0.0)
        khat = persist.tile([P, FC, T], bf16)
        # v + ones column (softmax normalizer Z rides as row 64 of attn@v)
        va = persist.tile([P, MKV, H, D + 1], bf16)
        nc.vector.memset(va[:, :, :, D], 1.0)
        yT = persist.tile([P, FC, TQ], bf16)

        # ---- phase A: projections + layernorm ----
        with contextlib.ExitStack() as pa:
            xkpool = pa.enter_context(tc.tile_pool(name="xkpool", bufs=1))
            wpool = pa.enter_context(tc.tile_pool(name="wpool", bufs=2))
            work = pa.enter_context(tc.tile_pool(name="work", bufs=1))
            ps = pa.enter_context(tc.tile_pool(name="psA", bufs=1, space="PSUM"))

            def ln_post(raw, S, Q2, dsts, gc, bc):
                # layernorm stats/apply for one [128, 512] half
                n = raw[:].free_size()
                mu = work.tile([2, n], bf16, tag="st", bufs=6)
                nc.scalar.activation(out=mu[:], in_=S, func=AF.Identity)
                mu2 = work.tile([2, n], f32, tag="st", bufs=6)
                nc.gpsimd.tensor_tensor(out=mu2[:], in0=mu[:], in1=mu[:],
                                        op=OP.mult)
                u = work.tile([2, n], f32, tag="st", bufs=6)
                nc.vector.scalar_tensor_tensor(
                    out=u[:], in0=Q2, scalar=1.0, in1=mu2[:],
                    op0=OP.mult, op1=OP.subtract)
                std = work.tile([2, n], f32, tag="st", bufs=6)
                nc.scalar.activation(out=std[:], in_=u[:], func=AF.Sqrt,
                                     bias=eps2[:], scale=1.0)
                r = work.tile([2, n], f32, tag="st", bufs=6)
                nc.vector.reciprocal_approx_fast(out=r[:], in_=std[:])
                r16 = work.tile([2, n], bf16, tag="st", bufs=6)
                nc.vector.tensor_copy(out=r16[:], in_=r[:])
                db = dr.tile([2, 2, n], bf16, tag="db", bufs=6)
                nc.sync.dma_start(out=db[:, 0, :], in_=mu[:])
                nc.sync.dma_start(out=db[:, 1, :], in_=r16[:])
                rb = work.tile([P, 2, n], bf16, tag="rb", bufs=4)
                dbap = db[:]
                bc_read(rb[0:64, :, :], dbap, 0, 64, [[n, 2], [1, n]])
                bc_read(rb[64:128, :, :], dbap, 2 * n, 64, [[n, 2], [1, n]])
                tmp = work.tile([P, n], bf16, tag="tmp", bufs=4)
                nc.vector.tensor_tensor(out=tmp[:], in0=raw[:],
                                        in1=rb[:, 0, :], op=OP.subtract)
                for (psl, dst) in dsts:
                    if affine:
                        tmp2 = work.tile([P, n], bf16, tag="tmp2", bufs=4)
                        nc.vector.tensor_tensor(out=tmp2[psl, :],
                                                in0=tmp[psl, :],
                                                in1=rb[psl, 1, :], op=OP.mult)
                        nc.vector.tensor_scalar(out=dst, in0=tmp2[psl, :],
                                                scalar1=gc[psl, 0:1],
                                                scalar2=bc[psl, 0:1],
                                                op0=OP.mult, op1=OP.add)
                    else:
                        nc.vector.tensor_tensor(out=dst, in0=tmp[psl, :],
                                                in1=rb[psl, 1, :], op=OP.mult)

            # q and k passes: transposed projections, pipelined stats
            pending = []
            xq_ctx = None
            xkv_sb = None
            for kind in ("q", "k"):
                tcols = TQ if kind == "q" else T
                fbase = 0 if kind == "q" else E
                if affine:
                    gc = gq_c if kind == "q" else gk_c
                    bc = bq_c if kind == "q" else bk_c
                else:
                    gc = bc = None
                if kind == "q":
                    xq_ctx = contextlib.ExitStack()
                    xqpool = xq_ctx.enter_context(
                        tc.tile_pool(name="xqpool", bufs=1))
                    xsb = xqpool.tile([P, KB, TQ], bf16, name="xq")
                    for j in range(2):
                        nc.sync.dma_start(
                            out=xsb[:, 4 * j:4 * (j + 1), :],
                            in_=xT_q[4 * j * P:4 * (j + 1) * P, :].rearrange(
                                "(kb p) t -> p kb t", p=P))
                else:
                    xsb = xkpool.tile([P, KB, T], bf16, name="xkv")
                    for j in range(4):
                        nc.sync.dma_start(
                            out=xsb[:, 2 * j:2 * (j + 1), :],
                            in_=xT_kv[2 * j * P:2 * (j + 1) * P, :].rearrange(
                                "(kb p) t -> p kb t", p=P))
                    xkv_sb = xsb
                for c in range(FC):
                    wch = wpool.tile([P, KB, P], bf16, name=f"w_{kind}{c}",
                                     tag="wqk", bufs=3)
                    nc.sync.dma_start(
                        out=wch[:],
                        in_=Wqkv[:, fbase + c * P:fbase + (c + 1) * P]
                        .rearrange("(kb p) f -> p kb f", p=P))
                    nnk = tcols // 512
                    pqs = [ps.tile([P, 512], f32, name=f"p_{kind}{c}_{nk}",
                                   tag="pqk", bufs=6) for nk in range(nnk)]
                    for kb in range(KB):
                        for nk in range(nnk):
                            nc.tensor.matmul(pqs[nk][:], wch[:, kb, :],
                                             xsb[:, kb,
                                                 nk * 512:(nk + 1) * 512],
                                             start=(kb == 0), stop=(kb == KB - 1))
                    for nk in range(nnk):
                        nsl = slice(nk * 512, (nk + 1) * 512)
                        pq = pqs[nk]
                        raw = work.tile([P, 512], bf16, tag="raw", bufs=3)
                        bias_col = bcol[:, (0 if kind == "q" else FC) + c:
                                        (0 if kind == "q" else FC) + c + 1]
                        nc.scalar.activation(out=raw[:], in_=pq[:],
                                             func=AF.Identity, bias=bias_col)
                        qsq = work.tile([P, 512], bf16, tag="qsq", bufs=3)
                        nc.vector.tensor_tensor(out=qsq[:], in0=raw[:],
                                                in1=raw[:], op=OP.mult)
                        st_ps = ps.tile([34, 512], f32, tag="stps", bufs=2)
                        nc.tensor.matmul(st_ps[0:2, :], ones_bd[:], raw[:],
                                         start=True, stop=True)
                        nc.tensor.matmul(st_ps[32:34, :], ones_bd[:], qsq[:],
                                         start=True, stop=True)
                        if kind == "q":
                            dsts = [(slice(0, 64), qhat[0:64, 0, c, nsl]),
                                    (slice(64, 128), qhat[64:128, 1, c, nsl])]
                        else:
                            dsts = [(slice(0, 128), khat[:, c, nsl])]
                        # run the non-PE tail of the previous half now
                        if pending:
                            ln_post(*pending.pop(0))
                        pending.append((raw, st_ps[0:2, :], st_ps[32:34, :],
                                        dsts, gc, bc))
                if kind == "q":
                    while pending:
                        ln_post(*pending.pop(0))
                    xq_ctx.close()
            while pending:
                ln_post(*pending.pop(0))

            # v pass: natural layout
            for c2 in range(2):
                wv = wpool.tile([P, KB, 512], bf16, name=f"w_v{c2}",
                                tag="wv", bufs=1)
                nc.sync.dma_start(
                    out=wv[:],
                    in_=Wqkv[:, 2 * E + c2 * 512:2 * E + (c2 + 1) * 512]
                    .rearrange("(kb p) f -> p kb f", p=P))
                for m in range(MKV):
                    pv = ps.tile([P, 512], f32, name=f"p_v{c2}_{m}",
                                 tag="pqk", bufs=6)
                    msl = slice(m * P, (m + 1) * P)
                    for kb in range(KB):
                        nc.tensor.matmul(pv[:], xkv_sb[:, kb, msl],
                                         wv[:, kb, :],
                                         start=(kb == 0), stop=(kb == KB - 1))
                    pv3 = pv[:].rearrange("p (h d) -> p h d", h=8)
                    hsl = slice(c2 * 8, (c2 + 1) * 8)
                    nc.scalar.activation(out=va[:, m, hsl, 0:D],
                                         in_=pv3[:], func=AF.Identity)

        late = top.enter_context(tc.tile_pool(name="late", bufs=1))
        wp_all = late.tile([P, KB, E], bf16)
        nc.sync.dma_start(out=wp_all[:],
                          in_=Wproj[:].rearrange("(kb p) f -> p kb f", p=P))

        if DEBUG:
            nc.sync.dma_start(out=dbg_q[:], in_=qhat[:])
            nc.sync.dma_start(out=dbg_k[:], in_=khat[:])
            nc.sync.dma_start(out=dbg_va[:], in_=va[:])

        # ---- phase C: attention, head pairs share the khat stationary ----
        with contextlib.ExitStack() as pc:
            cwork = pc.enter_context(tc.tile_pool(name="cwork", bufs=1))
            psc = pc.enter_context(tc.tile_pool(name="psC", bufs=1, space="PSUM"))
            LAG = 2
            for ch in range(FC):
                ys = [psc.tile([D + 1, TQ], f32, name=f"y_{ch}_{hh}",
                               tag="y", bufs=2) for hh in range(2)]
                pas = {}
                for tk in range(MKV + LAG):
                    if tk < MKV:
                        ksl = slice(tk * P, (tk + 1) * P)
                        ss = []
                        pp2 = []
                        for hh in range(2):
                            ss.append(psc.tile([P, TQ], f32,
                                               name=f"s_{ch}_{tk}_{hh}",
                                               tag="s", bufs=2))
                            pp2.append(cwork.tile([P, TQ], bf16,
                                                  name=f"pa_{ch}_{tk}_{hh}",
                                                  tag="pa", bufs=6))
                        for hh in range(2):
                            for nk in range(2):
                                nsl = slice(nk * 512, (nk + 1) * 512)
                                nc.tensor.matmul(
                                    ss[hh][:, nsl], khat[:, ch, ksl],
                                    qhat[:, hh, ch, nsl],
                                    start=True, stop=True)
                        for hh in range(2):
                            nc.scalar.activation(out=pp2[hh][:], in_=ss[hh][:],
                                                 func=AF.Exp, scale=SCALE)
                        pas[tk] = pp2
                    if tk >= LAG:
                        ppc = pas.pop(tk - LAG)
                        st = (tk - LAG == 0)
                        sp = (tk - LAG == MKV - 1)
                        for hh in range(2):
                            h = 2 * ch + hh
                            for nk in range(2):
                                nsl = slice(nk * 512, (nk + 1) * 512)
                                nc.tensor.matmul(ys[hh][:, nsl],
                                                 va[:, tk - LAG, h, :],
                                                 ppc[hh][:, nsl],
                                                 start=st, stop=sp)
                for hh in range(2):
                    r0 = hh * 64
                    y = ys[hh]
                    zrow = cwork.tile([1, TQ], f32, tag="zrow", bufs=2)
                    nc.vector.tensor_copy(out=zrow[:], in_=y[D:D + 1, :])
                    rz = cwork.tile([1, TQ], f32, tag="rz", bufs=2)
                    nc.vector.reciprocal_approx_fast(out=rz[:], in_=zrow[:])
                    zb = dr.tile([TQ], f32, tag="zb", bufs=2)
                    nc.sync.dma_start(out=zb[:], in_=rz[:])
                    rzb = cwork.tile([64, TQ], f32, tag="rzb", bufs=2)
                    bc_read(rzb[:], zb[:], 0, 64, [[1, TQ]])
                    nc.vector.tensor_tensor(out=yT[r0:r0 + 64, ch, :],
                                            in0=y[0:D, :], in1=rzb[:],
                                            op=OP.mult)

        if DEBUG:
            nc.sync.dma_start(out=dbg_y[:], in_=yT[:])

        # ---- phase D: output projection ----
        with contextlib.ExitStack() as pd:
            dwork = pd.enter_context(tc.tile_pool(name="dwork", bufs=1))
            psd = pd.enter_context(tc.tile_pool(name="psD", bufs=1, space="PSUM"))
            for m in range(TQ // P):
                msl = slice(m * P, (m + 1) * P)
                pos = [psd.tile([P, 512], f32, name=f"po_{m}_{nk}",
                                tag="po", bufs=4) for nk in range(2)]
                for kb in range(KB):
                    for nk in range(2):
                        nc.tensor.matmul(pos[nk][:], yT[:, kb, msl],
                                         wp_all[:, kb,
                                                nk * 512:(nk + 1) * 512],
                                         start=(kb == 0), stop=False)
                for nk in range(2):
                    nsl = slice(nk * 512, (nk + 1) * 512)
                    po = pos[nk]
                    nc.tensor.matmul(po[:], ones1[:], beff_row[:, nsl],
                                     start=False, stop=True)
                    osb = dwork.tile([P, 512], f32, tag="osb", bufs=4)
                    nc.scalar.activation(out=osb[:], in_=po[:],
                                         func=AF.Identity)
                    nc.sync.dma_start(out=out[msl, nsl], in_=osb[:])

    nc.finalize()
    return nc


def _get_nc(affine: bool):
    key = bool(affine)
    if key not in _BUILT:
        _BUILT[key] = _build_real(key)
    return _BUILT[key]


def kernel(x, Wqkv, bqkv, q_gamma, q_beta, k_gamma, k_beta, Wproj, bproj):
    from concourse.bass_utils import run_bass_kernel_spmd

    x = np.asarray(x, dtype=np.float32)
    Wqkv = np.asarray(Wqkv, dtype=np.float32)
    bqkv = np.asarray(bqkv, dtype=np.float32)
    Wproj = np.asarray(Wproj, dtype=np.float32)
    bproj = np.asarray(bproj, dtype=np.float32)
    q_gamma = np.asarray(q_gamma, dtype=np.float32)
    q_beta = np.asarray(q_beta, dtype=np.float32)
    k_gamma = np.asarray(k_gamma, dtype=np.float32)
    k_beta = np.asarray(k_beta, dtype=np.float32)

    affine = not (np.all(q_gamma == 1.0) and np.all(q_beta == 0.0)
                  and np.all(k_gamma == 1.0) and np.all(k_beta == 0.0))
    nc = _get_nc(affine)

    bf = ml_dtypes.bfloat16
    Wqkv_b = np.ascontiguousarray(Wqkv.astype(bf))
    Wproj_b = np.ascontiguousarray(Wproj.astype(bf))
    beff = (bqkv[2 * E:].astype(np.float64) @ Wproj.astype(np.float64)
            + bproj.astype(np.float64)).astype(np.float32).astype(bf)

    in_maps = []
    for c in range(NCORES):
        b, half = divmod(c, 2)
        xT_kv = np.ascontiguousarray(x[b].T.astype(bf))
        xT_q = np.ascontiguousarray(x[b, half * TQ:(half + 1) * TQ].T.astype(bf))
        m = {
            "xT_q": xT_q, "xT_kv": xT_kv,
            "Wqkv": Wqkv_b, "bqkv": bqkv, "beff": beff,
            "Wproj": Wproj_b,
        }
        if affine:
            m.update({"q_gamma": q_gamma, "q_beta": q_beta,
                      "k_gamma": k_gamma, "k_beta": k_beta})
        in_maps.append(m)

    global _last_in_maps
    _last_in_maps = in_maps
    res = run_bass_kernel_spmd(nc, in_maps, core_ids=list(range(NCORES)))
    y = np.empty((B, T, E), dtype=np.float32)
    for c in range(NCORES):
        b, half = divmod(c, 2)
        y[b, half * TQ:(half + 1) * TQ, :] = res.results[c]["out"]
    return y



# revision 13
# speedup vs baseline: 1.0297x; 1.0297x over previous
# Multi-head attention block (QKV proj + per-head q/k layernorm + softmax
# attention + output proj) on 8 Trainium2 NeuronCores.
#
# Sharding: data-parallel over (batch, query-half). Core c handles batch
# c//2, query tokens [ (c%2)*1024, (c%2+1)*1024 ). Each core computes K/V
# for its batch's full 2048 tokens; no cross-core communication, the host
# concatenates the 8 disjoint output chunks.
#
# On-device schedule (v2 — engine-balanced):
#   q,k are produced in [feature, token] layout (stationary = Wqkv block,
#   moving = xT). PSUM eviction + qkv bias runs on DVE (tensor_scalar_add
#   with a per-partition bias column), squares for the variance run on
#   GPSIMD, so the scalar (ACT) engine stays free for softmax exp.
#   LayerNorm stats (mean / E[x^2] over D=64 partition groups) come from
#   ones-column matmuls that land in ONE batched PSUM stats tile at
#   distinct partition offsets; they are post-processed in a handful of
#   batched ops (ACT Square, Sqrt; DVE sub/recip/pack), bounced via DRAM,
#   and broadcast-read back; the normalize applies in place on qhat/khat.
#   q is stored zero-padded per head ([q_head;0] / [0;q_head]) so score
#   matmuls contract over K=128 and share one stationary load. v is
#   natural-layout with a ones column so the softmax normalizer Z rides
#   attn@v as PSUM row 64. The v bias is folded into
#   beff = bv @ Wproj + bproj on the host.
#   Attention runs per (head-pair, query-512-half): scores -> one
#   [128,1024] exp per kv tile (both heads) -> attn@v into 1-bank
#   [65,512] PSUM accumulators (4 rotating bufs, so head-pair boundaries
#   never stall the PE and the HAM clock stays warm).
import contextlib

import numpy as np
import ml_dtypes

B, T, E = 4, 2048, 1024
H, D = 16, 64
P = 128
EPS = 1e-5
SCALE = 0.125  # 1/sqrt(D)
TQ = T // 2          # query tokens per core
KB = E // P          # contraction blocks
FC = E // P          # feature chunks for q/k (2 heads each)
MKV = T // P         # kv token tiles
NCORES = 8

_BUILT = {}
_last_in_maps = None
DEBUG = False


def _build_real(affine: bool, debug: bool = False):
    import concourse.bass as bass
    import concourse.bacc as bacc
    import concourse.tile as tile
    from concourse import mybir

    f32 = mybir.dt.float32
    bf16 = mybir.dt.bfloat16
    AF = mybir.ActivationFunctionType
    OP = mybir.AluOpType

    nc = bacc.Bacc("TRN2", target_bir_lowering=False)
    # xT_kv column order is [own query half | other half] (host permutes);
    # the q pass reads columns 0:TQ, attention is kv-order invariant.
    xT_kv = nc.declare_dram_parameter("xT_kv", [E, T], bf16, isOutput=False)
    Wqkv = nc.declare_dram_parameter("Wqkv", [E, 3 * E], bf16, isOutput=False)
    bqkv = nc.declare_dram_parameter("bqkv", [3 * E], f32, isOutput=False)
    beff = nc.declare_dram_parameter("beff", [E], bf16, isOutput=False)
    if affine:
        q_gamma = nc.declare_dram_parameter("q_gamma", [D], f32, isOutput=False)
        q_beta = nc.declare_dram_parameter("q_beta", [D], f32, isOutput=False)
        k_gamma = nc.declare_dram_parameter("k_gamma", [D], f32, isOutput=False)
        k_beta = nc.declare_dram_parameter("k_beta", [D], f32, isOutput=False)
    Wproj = nc.declare_dram_parameter("Wproj", [E, E], bf16, isOutput=False)
    out = nc.declare_dram_parameter("out", [TQ, E], f32, isOutput=True)
    if debug:
        dbg_q = nc.declare_dram_parameter("dbg_q", [P, 2, FC, TQ], bf16,
                                          isOutput=True)
        dbg_k = nc.declare_dram_parameter("dbg_k", [P, FC, T], bf16,
                                          isOutput=True)
        dbg_va = nc.declare_dram_parameter("dbg_va", [P, MKV, H, D + 1], bf16,
                                           isOutput=True)
        dbg_y = nc.declare_dram_parameter("dbg_y", [P, FC, TQ], bf16,
                                          isOutput=True)
        dbg_stq = nc.declare_dram_parameter("dbg_stq", [64, 2, 512], f32,
                                            isOutput=True)
        dbg_stk = nc.declare_dram_parameter("dbg_stk", [64, 2, 512], f32,
                                            isOutput=True)
        dbg_dbq = nc.declare_dram_parameter("dbg_dbq", [32, 2, 512], bf16,
                                            isOutput=True)
        dbg_dbk = nc.declare_dram_parameter("dbg_dbk", [64, 2, 512], bf16,
                                            isOutput=True)

    def bc_read(dst, tensor_ap, elem_off, ap_list):
        # broadcast-read from a DRAM bounce buffer with an explicit AP
        ap = bass.AP(tensor=tensor_ap.tensor,
                     offset=tensor_ap.offset + elem_off,
                     ap=ap_list)
        nc.gpsimd.dma_start(out=dst, in_=ap)

    with tile.TileContext(nc) as tc, contextlib.ExitStack() as top:
        const = top.enter_context(tc.tile_pool(name="const", bufs=1))
        persist = top.enter_context(tc.tile_pool(name="persist", bufs=1))
        dr = top.enter_context(tc.tile_pool(name="dr", bufs=1, space="DRAM"))

        ones1 = const.tile([1, P], bf16)
        nc.vector.memset(ones1[:], 1.0)
        ones_bd = const.tile([P, 2], bf16)
        nc.vector.memset(ones_bd[:], 0.0)
        nc.vector.memset(ones_bd[0:64, 0:1], 1.0 / 64.0)
        nc.vector.memset(ones_bd[64:128, 1:2], 1.0 / 64.0)
        bcol = const.tile([P, 16], f32)   # q/k bias, per-partition columns
        nc.sync.dma_start(out=bcol[:],
                          in_=bqkv[0:2 * E].rearrange("(c p) -> p c", p=P))
        beff_row = const.tile([1, E], bf16)
        nc.sync.dma_start(out=beff_row[:], in_=beff[:])
        eps_c = const.tile([P, 1], f32)
        nc.vector.memset(eps_c[:], EPS)
        if affine:
            gq_c = const.tile([P, 1], f32)
            bq_c = const.tile([P, 1], f32)
            gk_c = const.tile([P, 1], f32)
            bk_c = const.tile([P, 1], f32)
            for cc, src in ((gq_c, q_gamma), (bq_c, q_beta),
                            (gk_c, k_gamma), (bk_c, k_beta)):
                nc.sync.dma_start(out=cc[0:64, :], in_=src[:])
                nc.sync.dma_start(out=cc[64:128, :], in_=src[:])

        # qhat[:, 0] = [q_even; 0], qhat[:, 1] = [0; q_odd] (K=128 scores)
        qhat = persist.tile([P, 2, FC, TQ], bf16)
        nc.vector.memset(qhat[64:128, 0, :, :], 0.0)
        nc.vector.memset(qhat[0:64, 1, :, :], 0.0)
        khat = persist.tile([P, FC, T], bf16)
        # v + ones column (softmax normalizer Z rides as row 64 of attn@v)
        va = persist.tile([P, MKV, H, D + 1], bf16)
        nc.vector.memset(va[:, :, :, D], 1.0)
        yT = persist.tile([P, FC, TQ], bf16)

        # DRAM bounce buffers for the (mu, rstd) broadcast
        db_q = dr.tile([32, 2, 512], bf16)   # [2*combo+eo, {mu,rstd}, 512]
        db_k = dr.tile([64, 2, 512], bf16)

        # ---- phase A: projections + layernorm ----
        with contextlib.ExitStack() as pa:
            xkpool = pa.enter_context(tc.tile_pool(name="xkpool", bufs=1))
            wpool = pa.enter_context(tc.tile_pool(name="wpool", bufs=2))
            work = pa.enter_context(tc.tile_pool(name="work", bufs=1))
            ps = pa.enter_context(tc.tile_pool(name="psA", bufs=1, space="PSUM"))

            # batched SBUF stats, one (mu, sq) tile pair per 16-combo
            # group, rows [2i', 2i'+1] per combo (base-0 tiles everywhere:
            # dual-SBUF ops need equal partition bases)
            stats = {"q": [], "k": []}
            for _kn, _ng in (("q", 1), ("k", 2)):
                for _g in range(_ng):
                    _mu = work.tile([32, 512], f32, name=f"st{_kn}m{_g}",
                                    tag=f"st{_kn}m{_g}", bufs=1)
                    _sq = work.tile([32, 512], f32, name=f"st{_kn}s{_g}",
                                    tag=f"st{_kn}s{_g}", bufs=1)
                    stats[_kn].append((_mu, _sq))

            def emit_tail(kind, c, nk, pq):
                # evict+bias -> qhat/khat, square on DVE, stats matmuls
                # into a small PSUM tile (mean rows 0:2, sq rows 32:34),
                # gathered via ACT + scalar-queue DMAs (sync stays free
                # for the big weight/x loads).
                i = (c * 2 + nk) if kind == "q" else (c * 4 + nk)
                g, li = divmod(i, 16)
                nsl = slice(nk * 512, (nk + 1) * 512)
                bias_c = (0 if kind == "q" else FC) + c
                st_mu, st_sq = stats[kind][g]
                sm = ps.tile([34, 512], f32, tag="sm", bufs=4)
                qsq = work.tile([P, 512], bf16, tag="qsq", bufs=3)
                if kind == "q":
                    slots = (qhat[0:64, 0, c, nsl], qhat[64:128, 1, c, nsl])
                    nc.vector.tensor_scalar_add(out=slots[0], in0=pq[0:64, :],
                                                scalar1=bcol[0:64,
                                                             bias_c:bias_c + 1])
                    nc.vector.tensor_scalar_add(out=slots[1], in0=pq[64:128, :],
                                                scalar1=bcol[64:128,
                                                             bias_c:bias_c + 1])
                    nc.vector.tensor_tensor(out=qsq[0:64, :], in0=slots[0],
                                            in1=slots[0], op=OP.mult)
                    nc.vector.tensor_tensor(out=qsq[64:128, :], in0=slots[1],
                                            in1=slots[1], op=OP.mult)
                    # mean: accumulate the two padded slots (zeros elsewhere)
                    nc.tensor.matmul(sm[0:2, :], ones_bd[:],
                                     qhat[:, 0, c, nsl], start=True,
                                     stop=False)
                    nc.tensor.matmul(sm[0:2, :], ones_bd[:],
                                     qhat[:, 1, c, nsl], start=False,
                                     stop=True)
                else:
                    slot = khat[:, c, nsl]
                    nc.vector.tensor_scalar_add(out=slot, in0=pq[:],
                                                scalar1=bcol[:,
                                                             bias_c:bias_c + 1])
                    nc.vector.tensor_tensor(out=qsq[:], in0=slot, in1=slot,
                                            op=OP.mult)
                    nc.tensor.matmul(sm[0:2, :], ones_bd[:], slot,
                                     start=True, stop=True)
                nc.tensor.matmul(sm[32:34, :], ones_bd[:], qsq[:],
                                 start=True, stop=True)
                smsb = work.tile([34, 512], f32, tag="smsb", bufs=3)
                nc.scalar.activation(out=smsb[:], in_=sm[:], func=AF.Identity)
                nc.scalar.dma_start(out=st_mu[2 * li:2 * li + 2, :],
                                    in_=smsb[0:2, :])
                nc.scalar.dma_start(out=st_sq[2 * li:2 * li + 2, :],
                                    in_=smsb[32:34, :])

            def emit_post(kind, g):
                # batched layernorm stats for one 16-combo group:
                # var = E[x^2]-mu^2, rstd, pack, bounce to DRAM
                st_mu, st_sq = stats[kind][g]
                db = db_q if kind == "q" else db_k
                mu2 = work.tile([32, 512], f32, tag="post", bufs=4)
                nc.scalar.activation(out=mu2[:], in_=st_mu[:],
                                     func=AF.Square)
                var = work.tile([32, 512], f32, tag="post", bufs=4)
                nc.vector.tensor_tensor(out=var[:], in0=st_sq[:],
                                        in1=mu2[:], op=OP.subtract)
                std = work.tile([32, 512], f32, tag="post", bufs=4)
                nc.scalar.activation(out=std[:], in_=var[:], func=AF.Sqrt,
                                     bias=eps_c[0:32, :])
                rstd = work.tile([32, 512], f32, tag="post", bufs=4)
                nc.vector.reciprocal_approx_fast(out=rstd[:], in_=std[:])
                packed = work.tile([32, 2, 512], bf16, tag="pack", bufs=2)
                nc.vector.tensor_copy(out=packed[:, 0, :], in_=st_mu[:])
                nc.vector.tensor_copy(out=packed[:, 1, :], in_=rstd[:])
                nc.scalar.dma_start(out=db[32 * g:32 * (g + 1)], in_=packed[:])

            def emit_apply(kind, c, nk):
                # in-place (x - mu) * rstd on qhat/khat
                i = (c * 2 + nk) if kind == "q" else (c * 4 + nk)
                nsl = slice(nk * 512, (nk + 1) * 512)
                db = db_q if kind == "q" else db_k
                rb = work.tile([P, 2, 512], bf16, tag="rb", bufs=4)
                bc_read(rb[:], db[:], 2 * i * 1024,
                        [[1024, 2], [0, 64], [512, 2], [1, 512]])
                if kind == "q":
                    slots = ((qhat[0:64, 0, c, nsl], slice(0, 64)),
                             (qhat[64:128, 1, c, nsl], slice(64, 128)))
                else:
                    slots = ((khat[:, c, nsl], slice(0, 128)),)
                for slot, psl in slots:
                    nc.vector.tensor_tensor(out=slot, in0=slot,
                                            in1=rb[psl, 0, :], op=OP.subtract)
                    nc.vector.tensor_tensor(out=slot, in0=slot,
                                            in1=rb[psl, 1, :], op=OP.mult)
                    if affine:
                        gc = gq_c if kind == "q" else gk_c
                        bc = bq_c if kind == "q" else bk_c
                        nc.vector.tensor_scalar(out=slot, in0=slot,
                                                scalar1=gc[psl, 0:1],
                                                scalar2=bc[psl, 0:1],
                                                op0=OP.mult, op1=OP.add)

            # ---- q pass then k pass (q reads the kv x tile, cols 0:TQ) ----
            xkv_sb = xkpool.tile([P, KB, T], bf16, name="xkv")
            for j in range(8):
                nc.sync.dma_start(
                    out=xkv_sb[:, j:j + 1, :],
                    in_=xT_kv[j * P:(j + 1) * P, :].rearrange(
                        "(kb p) t -> p kb t", p=P))
            pending = []
            applies = []

            def pop_tail():
                # emit the lagged tail; when it completes a 16-combo group,
                # run that group's post and queue its normalize applies
                kind, c, nk, pq = pending.pop(0)
                emit_tail(kind, c, nk, pq)
                nnk_ = 2 if kind == "q" else 4
                i = c * nnk_ + nk
                if i % 16 == 15:
                    g = i // 16
                    emit_post(kind, g)
                    lo = g * 16
                    applies.extend(
                        (kind, j // nnk_, j % nnk_)
                        for j in range(lo, lo + 16))

            for kind in ("q", "k"):
                tcols = TQ if kind == "q" else T
                fbase = 0 if kind == "q" else E
                nnk = tcols // 512
                xsb = xkv_sb
                for c in range(FC):
                    wch = wpool.tile([P, KB, P], bf16, name=f"w_{kind}{c}",
                                     tag="wqk", bufs=3)
                    nc.sync.dma_start(
                        out=wch[:],
                        in_=Wqkv[:, fbase + c * P:fbase + (c + 1) * P]
                        .rearrange("(kb p) f -> p kb f", p=P))
                    for nk in range(nnk):
                        pq = ps.tile([P, 512], f32, name=f"p_{kind}{c}_{nk}",
                                     tag="pqk", bufs=4)
                        nsl = slice(nk * 512, (nk + 1) * 512)
                        for kb in range(KB):
                            nc.tensor.matmul(pq[:], wch[:, kb, :],
                                             xsb[:, kb, nsl],
                                             start=(kb == 0),
                                             stop=(kb == KB - 1))
                        if pending:
                            pop_tail()
                        pending.append((kind, c, nk, pq))
                        if applies:
                            emit_apply(*applies.pop(0))
                while pending:
                    pop_tail()
            # preload the exp table set while v runs (ACT is idle here)
            dummy = work.tile([1, 8], bf16, tag="dummy", bufs=1)
            nc.scalar.activation(out=dummy[:], in_=bcol[0:1, 0:8],
                                 func=AF.Exp)

            # ---- v pass: natural layout; k applies ride along ----
            for c2 in range(2):
                wv = wpool.tile([P, KB, 512], bf16, name=f"w_v{c2}",
                                tag="wv", bufs=1)
                nc.sync.dma_start(
                    out=wv[:],
                    in_=Wqkv[:, 2 * E + c2 * 512:2 * E + (c2 + 1) * 512]
                    .rearrange("(kb p) f -> p kb f", p=P))
                for m in range(MKV):
                    pv = ps.tile([P, 512], f32, name=f"p_v{c2}_{m}",
                                 tag="pqk", bufs=4)
                    msl = slice(m * P, (m + 1) * P)
                    for kb in range(KB):
                        nc.tensor.matmul(pv[:], xkv_sb[:, kb, msl],
                                         wv[:, kb, :],
                                         start=(kb == 0), stop=(kb == KB - 1))
                    pv3 = pv[:].rearrange("p (h d) -> p h d", h=8)
                    hsl = slice(c2 * 8, (c2 + 1) * 8)
                    nc.scalar.activation(out=va[:, m, hsl, 0:D],
                                         in_=pv3[:], func=AF.Identity)
                    if applies:
                        emit_apply(*applies.pop(0))
            while applies:
                emit_apply(*applies.pop(0))
            if debug:
                nc.sync.dma_start(out=dbg_stq[:, 0, :], in_=stats["q"][0][:])
                nc.sync.dma_start(out=dbg_stq[:, 1, :], in_=stats["q"][1][:])
                nc.sync.dma_start(out=dbg_stk[:, 0, :], in_=stats["k"][0][:])
                nc.sync.dma_start(out=dbg_stk[:, 1, :], in_=stats["k"][1][:])

        if debug:
            nc.sync.dma_start(out=dbg_q[:], in_=qhat[:])
            nc.sync.dma_start(out=dbg_k[:], in_=khat[:])
            nc.sync.dma_start(out=dbg_va[:], in_=va[:])
            nc.sync.dma_start(out=dbg_dbq[:], in_=db_q[:])
            nc.sync.dma_start(out=dbg_dbk[:], in_=db_k[:])

        late = top.enter_context(tc.tile_pool(name="late", bufs=1))
        wp_all = late.tile([P, KB, E], bf16)
        nc.sync.dma_start(out=wp_all[:],
                          in_=Wproj[:].rearrange("(kb p) f -> p kb f", p=P))

        # ---- phase C: attention per (head-pair, query-512-half) ----
        with contextlib.ExitStack() as pc:
            cwork = pc.enter_context(tc.tile_pool(name="cwork", bufs=1))
            psc = pc.enter_context(tc.tile_pool(name="psC", bufs=1,
                                                space="PSUM"))
            LAG = 2
            for ch in range(FC):
                for s in range(2):
                    ssl = slice(s * 512, (s + 1) * 512)
                    ys = [psc.tile([D + 1, 512], f32, name=f"y_{ch}{s}_{hh}",
                                   tag="y", bufs=4) for hh in range(2)]
                    pend = {}
                    for tk in range(MKV + LAG):
                        if tk < MKV:
                            ksl = slice(tk * P, (tk + 1) * P)
                            sst = psc.tile([P, 1024], f32,
                                           name=f"s_{ch}{s}_{tk}",
                                           tag="s", bufs=2)
                            for hh in range(2):
                                nc.tensor.matmul(
                                    sst[:, hh * 512:(hh + 1) * 512],
                                    khat[:, ch, ksl],
                                    qhat[:, hh, ch, ssl],
                                    start=True, stop=True)
                            ppt = cwork.tile([P, 1024], bf16,
                                             name=f"pa_{ch}{s}_{tk}",
                                             tag="pa", bufs=6)
                            nc.scalar.activation(out=ppt[:], in_=sst[:],
                                                 func=AF.Exp, scale=SCALE)
                            pend[tk] = ppt
                        if tk >= LAG:
                            t = tk - LAG
                            ppc = pend.pop(t)
                            st_ = (t == 0)
                            sp_ = (t == MKV - 1)
                            for hh in range(2):
                                nc.tensor.matmul(
                                    ys[hh][:],
                                    va[:, t, 2 * ch + hh, :],
                                    ppc[:, hh * 512:(hh + 1) * 512],
                                    start=st_, stop=sp_)
                    # softmax normalize: yT = y * (1/Z) broadcast over D rows
                    zbt = dr.tile([2, 512], f32, tag="zb", bufs=4)
                    for hh in range(2):
                        # copy Z to a base-0 tile first: the custom-DVE
                        # reciprocal ignores a non-zero partition base
                        zrow = cwork.tile([1, 512], f32, tag="zrow", bufs=8)
                        nc.vector.tensor_copy(out=zrow[:],
                                              in_=ys[hh][D:D + 1, :])
                        rzh = cwork.tile([1, 512], f32, tag="rz", bufs=8)
                        nc.vector.reciprocal_approx_fast(
                            out=rzh[:], in_=zrow[:])
                        nc.sync.dma_start(out=zbt[hh:hh + 1, :], in_=rzh[:])
                    for hh in range(2):
                        r0 = hh * 64
                        rzbh = cwork.tile([64, 512], f32, tag="rzb", bufs=8)
                        bc_read(rzbh[:], zbt[:], hh * 512,
                                [[0, 64], [1, 512]])
                        nc.vector.tensor_tensor(out=yT[r0:r0 + 64, ch, ssl],
                                                in0=ys[hh][0:D, :],
                                                in1=rzbh[:],
                                                op=OP.mult)

        if debug:
            nc.sync.dma_start(out=dbg_y[:], in_=yT[:])

        # ---- phase D: output projection ----
        with contextlib.ExitStack() as pd:
            dwork = pd.enter_context(tc.tile_pool(name="dwork", bufs=1))
            psd = pd.enter_context(tc.tile_pool(name="psD", bufs=1,
                                                space="PSUM"))
            for m in range(TQ // P):
                msl = slice(m * P, (m + 1) * P)
                pos = [psd.tile([P, 512], f32, name=f"po_{m}_{nk}",
                                tag="po", bufs=4) for nk in range(2)]
                for kb in range(KB):
                    for nk in range(2):
                        nc.tensor.matmul(pos[nk][:], yT[:, kb, msl],
                                         wp_all[:, kb,
                                                nk * 512:(nk + 1) * 512],
                                         start=(kb == 0), stop=False)
                for nk in range(2):
                    nsl = slice(nk * 512, (nk + 1) * 512)
                    po = pos[nk]
                    nc.tensor.matmul(po[:], ones1[:], beff_row[:, nsl],
                                     start=False, stop=True)
                    osb = dwork.tile([P, 512], f32, tag="osb", bufs=4)
                    nc.scalar.activation(out=osb[:], in_=po[:],
                                         func=AF.Identity)
                    nc.sync.dma_start(out=out[msl, nsl], in_=osb[:])

    nc.finalize()
    return nc


def _get_nc(affine: bool):
    key = (bool(affine), DEBUG)
    if key not in _BUILT:
        _BUILT[key] = _build_real(bool(affine), DEBUG)
    return _BUILT[key]


def kernel(x, Wqkv, bqkv, q_gamma, q_beta, k_gamma, k_beta, Wproj, bproj):
    from concourse.bass_utils import run_bass_kernel_spmd

    x = np.asarray(x, dtype=np.float32)
    Wqkv = np.asarray(Wqkv, dtype=np.float32)
    bqkv = np.asarray(bqkv, dtype=np.float32)
    Wproj = np.asarray(Wproj, dtype=np.float32)
    bproj = np.asarray(bproj, dtype=np.float32)
    q_gamma = np.asarray(q_gamma, dtype=np.float32)
    q_beta = np.asarray(q_beta, dtype=np.float32)
    k_gamma = np.asarray(k_gamma, dtype=np.float32)
    k_beta = np.asarray(k_beta, dtype=np.float32)

    affine = not (np.all(q_gamma == 1.0) and np.all(q_beta == 0.0)
                  and np.all(k_gamma == 1.0) and np.all(k_beta == 0.0))
    nc = _get_nc(affine)

    bf = ml_dtypes.bfloat16
    Wqkv_b = np.ascontiguousarray(Wqkv.astype(bf))
    Wproj_b = np.ascontiguousarray(Wproj.astype(bf))
    beff = (bqkv[2 * E:].astype(np.float64) @ Wproj.astype(np.float64)
            + bproj.astype(np.float64)).astype(np.float32).astype(bf)

    in_maps = []
    for c in range(NCORES):
        b, half = divmod(c, 2)
        # kv token order = [own query half | other half]; attention output
        # is invariant to kv permutation and the q pass reads cols 0:TQ.
        xb = np.roll(x[b], -half * TQ, axis=0) if half else x[b]
        xT_kv = np.ascontiguousarray(xb.T.astype(bf))
        m = {
            "xT_kv": xT_kv,
            "Wqkv": Wqkv_b, "bqkv": bqkv, "beff": beff,
            "Wproj": Wproj_b,
        }
        if affine:
            m.update({"q_gamma": q_gamma, "q_beta": q_beta,
                      "k_gamma": k_gamma, "k_beta": k_beta})
        in_maps.append(m)

    global _last_in_maps
    _last_in_maps = in_maps
    res = run_bass_kernel_spmd(nc, in_maps, core_ids=list(range(NCORES)))
    y = np.empty((B, T, E), dtype=np.float32)
    for c in range(NCORES):
        b, half = divmod(c, 2)
        y[b, half * TQ:(half + 1) * TQ, :] = res.results[c]["out"]
    return y


# revision 14
# speedup vs baseline: 1.2143x; 1.1793x over previous
# Multi-head attention block (QKV proj + per-head q/k layernorm + softmax
# attention + output proj) on 8 Trainium2 NeuronCores.
#
# Sharding: data-parallel over (batch, query-half). Core c handles batch
# c//2, query tokens [ (c%2)*1024, (c%2+1)*1024 ). Each core computes K/V
# for its batch's full 2048 tokens; no cross-core communication, the host
# concatenates the 8 disjoint output chunks.
#
# On-device schedule (v2 — engine-balanced):
#   q,k are produced in [feature, token] layout (stationary = Wqkv block,
#   moving = xT). PSUM eviction + qkv bias runs on DVE (tensor_scalar_add
#   with a per-partition bias column), squares for the variance run on
#   GPSIMD, so the scalar (ACT) engine stays free for softmax exp.
#   LayerNorm stats (mean / E[x^2] over D=64 partition groups) come from
#   ones-column matmuls that land in ONE batched PSUM stats tile at
#   distinct partition offsets; they are post-processed in a handful of
#   batched ops (ACT Square, Sqrt; DVE sub/recip/pack), bounced via DRAM,
#   and broadcast-read back; the normalize applies in place on qhat/khat.
#   q is stored zero-padded per head ([q_head;0] / [0;q_head]) so score
#   matmuls contract over K=128 and share one stationary load. v is
#   natural-layout with a ones column so the softmax normalizer Z rides
#   attn@v as PSUM row 64. The v bias is folded into
#   beff = bv @ Wproj + bproj on the host.
#   Attention runs per (head-pair, query-512-half): scores -> one
#   [128,1024] exp per kv tile (both heads) -> attn@v into 1-bank
#   [65,512] PSUM accumulators (4 rotating bufs, so head-pair boundaries
#   never stall the PE and the HAM clock stays warm).
import contextlib

import numpy as np
import ml_dtypes

B, T, E = 4, 2048, 1024
H, D = 16, 64
P = 128
EPS = 1e-5
SCALE = 0.125  # 1/sqrt(D)
TQ = T // 2          # query tokens per core
KB = E // P          # contraction blocks
FC = E // P          # feature chunks for q/k (2 heads each)
MKV = T // P         # kv token tiles
NCORES = 8

_BUILT = {}
_last_in_maps = None
DEBUG = False


def _build_real(affine: bool, debug: bool = False):
    import concourse.bass as bass
    import concourse.bacc as bacc
    import concourse.tile as tile
    from concourse import mybir

    f32 = mybir.dt.float32
    bf16 = mybir.dt.bfloat16
    AF = mybir.ActivationFunctionType
    OP = mybir.AluOpType

    nc = bacc.Bacc("TRN2", target_bir_lowering=False)
    # xT_kv column order is [own query half | other half] (host permutes);
    # the q pass reads columns 0:TQ, attention is kv-order invariant.
    xT_kv = nc.declare_dram_parameter("xT_kv", [E, T], bf16, isOutput=False)
    Wqkv = nc.declare_dram_parameter("Wqkv", [E, 3 * E], bf16, isOutput=False)
    bqkv = nc.declare_dram_parameter("bqkv", [3 * E], f32, isOutput=False)
    beff = nc.declare_dram_parameter("beff", [E], bf16, isOutput=False)
    if affine:
        q_gamma = nc.declare_dram_parameter("q_gamma", [D], f32, isOutput=False)
        q_beta = nc.declare_dram_parameter("q_beta", [D], f32, isOutput=False)
        k_gamma = nc.declare_dram_parameter("k_gamma", [D], f32, isOutput=False)
        k_beta = nc.declare_dram_parameter("k_beta", [D], f32, isOutput=False)
    Wproj = nc.declare_dram_parameter("Wproj", [E, E], bf16, isOutput=False)
    out = nc.declare_dram_parameter("out", [TQ, E], f32, isOutput=True)
    if debug:
        dbg_q = nc.declare_dram_parameter("dbg_q", [P, 2, FC, TQ], bf16,
                                          isOutput=True)
        dbg_k = nc.declare_dram_parameter("dbg_k", [P, FC, T], bf16,
                                          isOutput=True)
        dbg_va = nc.declare_dram_parameter("dbg_va", [P, MKV, H, D + 1], bf16,
                                           isOutput=True)
        dbg_y = nc.declare_dram_parameter("dbg_y", [P, FC, TQ], bf16,
                                          isOutput=True)
        dbg_stq = nc.declare_dram_parameter("dbg_stq", [64, 2, 512], f32,
                                            isOutput=True)
        dbg_stk = nc.declare_dram_parameter("dbg_stk", [64, 2, 512], f32,
                                            isOutput=True)
        dbg_dbq = nc.declare_dram_parameter("dbg_dbq", [32, 2, 512], bf16,
                                            isOutput=True)
        dbg_dbk = nc.declare_dram_parameter("dbg_dbk", [64, 2, 512], bf16,
                                            isOutput=True)

    def bc_read(dst, tensor_ap, elem_off, ap_list):
        # broadcast-read from a DRAM bounce buffer with an explicit AP
        ap = bass.AP(tensor=tensor_ap.tensor,
                     offset=tensor_ap.offset + elem_off,
                     ap=ap_list)
        nc.gpsimd.dma_start(out=dst, in_=ap)

    with tile.TileContext(nc) as tc, contextlib.ExitStack() as top:
        const = top.enter_context(tc.tile_pool(name="const", bufs=1))
        persist = top.enter_context(tc.tile_pool(name="persist", bufs=1))
        dr = top.enter_context(tc.tile_pool(name="dr", bufs=1, space="DRAM"))

        ones1 = const.tile([1, P], bf16)
        nc.vector.memset(ones1[:], 1.0)
        ones_bd = const.tile([P, 2], bf16)
        nc.vector.memset(ones_bd[:], 0.0)
        nc.vector.memset(ones_bd[0:64, 0:1], 1.0 / 64.0)
        nc.vector.memset(ones_bd[64:128, 1:2], 1.0 / 64.0)
        bcol = const.tile([P, 16], f32)   # q/k bias, per-partition columns
        nc.sync.dma_start(out=bcol[:],
                          in_=bqkv[0:2 * E].rearrange("(c p) -> p c", p=P))
        beff_row = const.tile([1, E], bf16)
        nc.sync.dma_start(out=beff_row[:], in_=beff[:])
        eps_c = const.tile([P, 1], f32)
        nc.vector.memset(eps_c[:], EPS)
        if affine:
            gq_c = const.tile([P, 1], f32)
            bq_c = const.tile([P, 1], f32)
            gk_c = const.tile([P, 1], f32)
            bk_c = const.tile([P, 1], f32)
            for cc, src in ((gq_c, q_gamma), (bq_c, q_beta),
                            (gk_c, k_gamma), (bk_c, k_beta)):
                nc.sync.dma_start(out=cc[0:64, :], in_=src[:])
                nc.sync.dma_start(out=cc[64:128, :], in_=src[:])

        # qhat[:, 0] = [q_even; 0], qhat[:, 1] = [0; q_odd] (K=128 scores)
        qhat = persist.tile([P, 2, FC, TQ], bf16)
        nc.vector.memset(qhat[64:128, 0, :, :], 0.0)
        nc.vector.memset(qhat[0:64, 1, :, :], 0.0)
        khat = persist.tile([P, FC, T], bf16)
        # v + ones column (softmax normalizer Z rides as row 64 of attn@v)
        va = persist.tile([P, MKV, H, D + 1], bf16)
        nc.vector.memset(va[:, :, :, D], 1.0)
        yT = persist.tile([P, FC, TQ], bf16)

        # DRAM bounce buffers for the (mu, rstd) broadcast
        db_q = dr.tile([32, 2, 512], bf16)   # [2*combo+eo, {mu,rstd}, 512]
        db_k = dr.tile([64, 2, 512], bf16)

        # ---- phase A: projections + layernorm ----
        with contextlib.ExitStack() as pa:
            xkpool = pa.enter_context(tc.tile_pool(name="xkpool", bufs=1))
            wpool = pa.enter_context(tc.tile_pool(name="wpool", bufs=2))
            work = pa.enter_context(tc.tile_pool(name="work", bufs=1))
            ps = pa.enter_context(tc.tile_pool(name="psA", bufs=1, space="PSUM"))

            # batched SBUF stats, one (mu, sq) tile pair per 16-combo
            # group, rows [2i', 2i'+1] per combo (base-0 tiles everywhere:
            # dual-SBUF ops need equal partition bases)
            stats = {"q": [], "k": []}
            for _kn, _ng in (("q", 1), ("k", 2)):
                for _g in range(_ng):
                    _mu = work.tile([32, 512], f32, name=f"st{_kn}m{_g}",
                                    tag=f"st{_kn}m{_g}", bufs=1)
                    _sq = work.tile([32, 512], f32, name=f"st{_kn}s{_g}",
                                    tag=f"st{_kn}s{_g}", bufs=1)
                    stats[_kn].append((_mu, _sq))

            def emit_tail(kind, c, nk, pq):
                # evict+bias -> qhat/khat, square on DVE, stats matmuls
                # into a small PSUM tile (mean rows 0:2, sq rows 32:34),
                # gathered via ACT + scalar-queue DMAs (sync stays free
                # for the big weight/x loads).
                i = (c * 2 + nk) if kind == "q" else (c * 4 + nk)
                g, li = divmod(i, 16)
                nsl = slice(nk * 512, (nk + 1) * 512)
                bias_c = (0 if kind == "q" else FC) + c
                st_mu, st_sq = stats[kind][g]
                sm = ps.tile([34, 512], f32, tag="sm", bufs=4)
                qsq = work.tile([P, 512], bf16, tag="qsq", bufs=3)
                if kind == "q":
                    slots = (qhat[0:64, 0, c, nsl], qhat[64:128, 1, c, nsl])
                    nc.vector.tensor_scalar_add(out=slots[0], in0=pq[0:64, :],
                                                scalar1=bcol[0:64,
                                                             bias_c:bias_c + 1])
                    nc.vector.tensor_scalar_add(out=slots[1], in0=pq[64:128, :],
                                                scalar1=bcol[64:128,
                                                             bias_c:bias_c + 1])
                    nc.vector.tensor_tensor(out=qsq[0:64, :], in0=slots[0],
                                            in1=slots[0], op=OP.mult)
                    nc.vector.tensor_tensor(out=qsq[64:128, :], in0=slots[1],
                                            in1=slots[1], op=OP.mult)
                    # mean: accumulate the two padded slots (zeros elsewhere)
                    nc.tensor.matmul(sm[0:2, :], ones_bd[:],
                                     qhat[:, 0, c, nsl], start=True,
                                     stop=False)
                    nc.tensor.matmul(sm[0:2, :], ones_bd[:],
                                     qhat[:, 1, c, nsl], start=False,
                                     stop=True)
                else:
                    slot = khat[:, c, nsl]
                    nc.vector.tensor_scalar_add(out=slot, in0=pq[:],
                                                scalar1=bcol[:,
                                                             bias_c:bias_c + 1])
                    nc.vector.tensor_tensor(out=qsq[:], in0=slot, in1=slot,
                                            op=OP.mult)
                    nc.tensor.matmul(sm[0:2, :], ones_bd[:], slot,
                                     start=True, stop=True)
                nc.tensor.matmul(sm[32:34, :], ones_bd[:], qsq[:],
                                 start=True, stop=True)
                smsb = work.tile([34, 512], f32, tag="smsb", bufs=3)
                nc.scalar.activation(out=smsb[:], in_=sm[:], func=AF.Identity)
                nc.scalar.dma_start(out=st_mu[2 * li:2 * li + 2, :],
                                    in_=smsb[0:2, :])
                nc.scalar.dma_start(out=st_sq[2 * li:2 * li + 2, :],
                                    in_=smsb[32:34, :])

            def emit_post(kind, g):
                # batched layernorm stats for one 16-combo group:
                # var = E[x^2]-mu^2, rstd, pack, bounce to DRAM
                st_mu, st_sq = stats[kind][g]
                db = db_q if kind == "q" else db_k
                mu2 = work.tile([32, 512], f32, tag="post", bufs=4)
                nc.scalar.activation(out=mu2[:], in_=st_mu[:],
                                     func=AF.Square)
                var = work.tile([32, 512], f32, tag="post", bufs=4)
                nc.vector.tensor_tensor(out=var[:], in0=st_sq[:],
                                        in1=mu2[:], op=OP.subtract)
                std = work.tile([32, 512], f32, tag="post", bufs=4)
                nc.scalar.activation(out=std[:], in_=var[:], func=AF.Sqrt,
                                     bias=eps_c[0:32, :])
                rstd = work.tile([32, 512], f32, tag="post", bufs=4)
                nc.vector.reciprocal_approx_fast(out=rstd[:], in_=std[:])
                packed = work.tile([32, 2, 512], bf16, tag="pack", bufs=2)
                nc.vector.tensor_copy(out=packed[:, 0, :], in_=st_mu[:])
                nc.vector.tensor_copy(out=packed[:, 1, :], in_=rstd[:])
                nc.scalar.dma_start(out=db[32 * g:32 * (g + 1)], in_=packed[:])

            def emit_apply_fetch(kind, c, nk):
                # broadcast-read (mu, rstd) for one combo; issued a few
                # slots ahead so the DVE apply never blocks on the DMA
                i = (c * 2 + nk) if kind == "q" else (c * 4 + nk)
                db = db_q if kind == "q" else db_k
                rb = work.tile([P, 2, 512], bf16, tag="rb", bufs=4)
                bc_read(rb[:], db[:], 2 * i * 1024,
                        [[1024, 2], [0, 64], [512, 2], [1, 512]])
                return (kind, c, nk, rb)

            def emit_apply(kind, c, nk, rb):
                # in-place (x - mu) * rstd on qhat/khat
                nsl = slice(nk * 512, (nk + 1) * 512)
                if kind == "q":
                    slots = ((qhat[0:64, 0, c, nsl], slice(0, 64)),
                             (qhat[64:128, 1, c, nsl], slice(64, 128)))
                else:
                    slots = ((khat[:, c, nsl], slice(0, 128)),)
                for slot, psl in slots:
                    nc.vector.tensor_tensor(out=slot, in0=slot,
                                            in1=rb[psl, 0, :], op=OP.subtract)
                    nc.vector.tensor_tensor(out=slot, in0=slot,
                                            in1=rb[psl, 1, :], op=OP.mult)
                    if affine:
                        gc = gq_c if kind == "q" else gk_c
                        bc = bq_c if kind == "q" else bk_c
                        nc.vector.tensor_scalar(out=slot, in0=slot,
                                                scalar1=gc[psl, 0:1],
                                                scalar2=bc[psl, 0:1],
                                                op0=OP.mult, op1=OP.add)

            # ---- q pass then k pass (q reads the kv x tile, cols 0:TQ) ----
            xkv_sb = xkpool.tile([P, KB, T], bf16, name="xkv")
            for j in range(8):
                nc.sync.dma_start(
                    out=xkv_sb[:, j:j + 1, :],
                    in_=xT_kv[j * P:(j + 1) * P, :].rearrange(
                        "(kb p) t -> p kb t", p=P))
            pending = []
            applies = []
            fetched = []

            def pump_applies():
                # prefetch up to 3 rb broadcasts ahead, apply the oldest
                while applies and len(fetched) < 3:
                    fetched.append(emit_apply_fetch(*applies.pop(0)))
                if fetched:
                    emit_apply(*fetched.pop(0))

            def pop_tail():
                # emit the lagged tail; when it completes a 16-combo group,
                # run that group's post and queue its normalize applies
                kind, c, nk, pq = pending.pop(0)
                emit_tail(kind, c, nk, pq)
                nnk_ = 2 if kind == "q" else 4
                i = c * nnk_ + nk
                if i % 16 == 15:
                    g = i // 16
                    emit_post(kind, g)
                    lo = g * 16
                    applies.extend(
                        (kind, j // nnk_, j % nnk_)
                        for j in range(lo, lo + 16))

            for kind in ("q", "k"):
                tcols = TQ if kind == "q" else T
                fbase = 0 if kind == "q" else E
                nnk = tcols // 512
                xsb = xkv_sb
                for c in range(FC):
                    wch = wpool.tile([P, KB, P], bf16, name=f"w_{kind}{c}",
                                     tag="wqk", bufs=3)
                    nc.sync.dma_start(
                        out=wch[:],
                        in_=Wqkv[:, fbase + c * P:fbase + (c + 1) * P]
                        .rearrange("(kb p) f -> p kb f", p=P))
                    for nk in range(nnk):
                        pq = ps.tile([P, 512], f32, name=f"p_{kind}{c}_{nk}",
                                     tag="pqk", bufs=4)
                        nsl = slice(nk * 512, (nk + 1) * 512)
                        for kb in range(KB):
                            nc.tensor.matmul(pq[:], wch[:, kb, :],
                                             xsb[:, kb, nsl],
                                             start=(kb == 0),
                                             stop=(kb == KB - 1))
                        if pending:
                            pop_tail()
                        pending.append((kind, c, nk, pq))
                        pump_applies()
                while pending:
                    pop_tail()
            # preload the exp table set while v runs (ACT is idle here)
            dummy = work.tile([1, 8], bf16, tag="dummy", bufs=1)
            nc.scalar.activation(out=dummy[:], in_=bcol[0:1, 0:8],
                                 func=AF.Exp)

            # ---- v pass: natural layout; k applies ride along ----
            for c2 in range(2):
                wv = wpool.tile([P, KB, 512], bf16, name=f"w_v{c2}",
                                tag="wv", bufs=1)
                nc.sync.dma_start(
                    out=wv[:],
                    in_=Wqkv[:, 2 * E + c2 * 512:2 * E + (c2 + 1) * 512]
                    .rearrange("(kb p) f -> p kb f", p=P))
                for m in range(MKV):
                    pv = ps.tile([P, 512], f32, name=f"p_v{c2}_{m}",
                                 tag="pqk", bufs=4)
                    msl = slice(m * P, (m + 1) * P)
                    for kb in range(KB):
                        nc.tensor.matmul(pv[:], xkv_sb[:, kb, msl],
                                         wv[:, kb, :],
                                         start=(kb == 0), stop=(kb == KB - 1))
                    pv3 = pv[:].rearrange("p (h d) -> p h d", h=8)
                    hsl = slice(c2 * 8, (c2 + 1) * 8)
                    nc.scalar.activation(out=va[:, m, hsl, 0:D],
                                         in_=pv3[:], func=AF.Identity)
                    pump_applies()
            while applies or fetched:
                pump_applies()
            if debug:
                nc.sync.dma_start(out=dbg_stq[:, 0, :], in_=stats["q"][0][:])
                nc.sync.dma_start(out=dbg_stq[:, 1, :], in_=stats["q"][1][:])
                nc.sync.dma_start(out=dbg_stk[:, 0, :], in_=stats["k"][0][:])
                nc.sync.dma_start(out=dbg_stk[:, 1, :], in_=stats["k"][1][:])

        if debug:
            nc.sync.dma_start(out=dbg_q[:], in_=qhat[:])
            nc.sync.dma_start(out=dbg_k[:], in_=khat[:])
            nc.sync.dma_start(out=dbg_va[:], in_=va[:])
            nc.sync.dma_start(out=dbg_dbq[:], in_=db_q[:])
            nc.sync.dma_start(out=dbg_dbk[:], in_=db_k[:])

        late = top.enter_context(tc.tile_pool(name="late", bufs=1))
        wp_all = late.tile([P, KB, E], bf16)
        nc.sync.dma_start(out=wp_all[:],
                          in_=Wproj[:].rearrange("(kb p) f -> p kb f", p=P))

        # ---- phase C: attention per (head-pair, query-512-half) ----
        with contextlib.ExitStack() as pc:
            cwork = pc.enter_context(tc.tile_pool(name="cwork", bufs=1))
            psc = pc.enter_context(tc.tile_pool(name="psC", bufs=1,
                                                space="PSUM"))
            LAG = 2
            for ch in range(FC):
                for s in range(2):
                    ssl = slice(s * 512, (s + 1) * 512)
                    ys = [psc.tile([D + 1, 512], f32, name=f"y_{ch}{s}_{hh}",
                                   tag="y", bufs=4) for hh in range(2)]
                    pend = {}
                    for tk in range(MKV + LAG):
                        if tk < MKV:
                            ksl = slice(tk * P, (tk + 1) * P)
                            sst = psc.tile([P, 1024], f32,
                                           name=f"s_{ch}{s}_{tk}",
                                           tag="s", bufs=2)
                            for hh in range(2):
                                nc.tensor.matmul(
                                    sst[:, hh * 512:(hh + 1) * 512],
                                    khat[:, ch, ksl],
                                    qhat[:, hh, ch, ssl],
                                    start=True, stop=True)
                            ppt = cwork.tile([P, 1024], bf16,
                                             name=f"pa_{ch}{s}_{tk}",
                                             tag="pa", bufs=6)
                            nc.scalar.activation(out=ppt[:], in_=sst[:],
                                                 func=AF.Exp, scale=SCALE)
                            pend[tk] = ppt
                        if tk >= LAG:
                            t = tk - LAG
                            ppc = pend.pop(t)
                            st_ = (t == 0)
                            sp_ = (t == MKV - 1)
                            for hh in range(2):
                                nc.tensor.matmul(
                                    ys[hh][:],
                                    va[:, t, 2 * ch + hh, :],
                                    ppc[:, hh * 512:(hh + 1) * 512],
                                    start=st_, stop=sp_)
                    # softmax normalize: yT = y * (1/Z) broadcast over D rows
                    zbt = dr.tile([2, 512], f32, tag="zb", bufs=4)
                    for hh in range(2):
                        # copy Z to a base-0 tile first: the custom-DVE
                        # reciprocal ignores a non-zero partition base
                        zrow = cwork.tile([1, 512], f32, tag="zrow", bufs=8)
                        nc.vector.tensor_copy(out=zrow[:],
                                              in_=ys[hh][D:D + 1, :])
                        rzh = cwork.tile([1, 512], f32, tag="rz", bufs=8)
                        nc.vector.reciprocal_approx_fast(
                            out=rzh[:], in_=zrow[:])
                        nc.sync.dma_start(out=zbt[hh:hh + 1, :], in_=rzh[:])
                    for hh in range(2):
                        r0 = hh * 64
                        rzbh = cwork.tile([64, 512], f32, tag="rzb", bufs=8)
                        bc_read(rzbh[:], zbt[:], hh * 512,
                                [[0, 64], [1, 512]])
                        nc.vector.tensor_tensor(out=yT[r0:r0 + 64, ch, ssl],
                                                in0=ys[hh][0:D, :],
                                                in1=rzbh[:],
                                                op=OP.mult)

        if debug:
            nc.sync.dma_start(out=dbg_y[:], in_=yT[:])

        # ---- phase D: output projection ----
        with contextlib.ExitStack() as pd:
            dwork = pd.enter_context(tc.tile_pool(name="dwork", bufs=1))
            psd = pd.enter_context(tc.tile_pool(name="psD", bufs=1,
                                                space="PSUM"))
            for m in range(TQ // P):
                msl = slice(m * P, (m + 1) * P)
                pos = [psd.tile([P, 512], f32, name=f"po_{m}_{nk}",
                                tag="po", bufs=4) for nk in range(2)]
                for kb in range(KB):
                    for nk in range(2):
                        nc.tensor.matmul(pos[nk][:], yT[:, kb, msl],
                                         wp_all[:, kb,
                                                nk * 512:(nk + 1) * 512],
                                         start=(kb == 0), stop=False)
                for nk in range(2):
                    nsl = slice(nk * 512, (nk + 1) * 512)
                    po = pos[nk]
                    nc.tensor.matmul(po[:], ones1[:], beff_row[:, nsl],
                                     start=False, stop=True)
                    osb = dwork.tile([P, 512], f32, tag="osb", bufs=4)
                    nc.scalar.activation(out=osb[:], in_=po[:],
                                         func=AF.Identity)
                    nc.sync.dma_start(out=out[msl, nsl], in_=osb[:])

    nc.finalize()
    return nc


def _get_nc(affine: bool):
    key = (bool(affine), DEBUG)
    if key not in _BUILT:
        _BUILT[key] = _build_real(bool(affine), DEBUG)
    return _BUILT[key]


def kernel(x, Wqkv, bqkv, q_gamma, q_beta, k_gamma, k_beta, Wproj, bproj):
    from concourse.bass_utils import run_bass_kernel_spmd

    x = np.asarray(x, dtype=np.float32)
    Wqkv = np.asarray(Wqkv, dtype=np.float32)
    bqkv = np.asarray(bqkv, dtype=np.float32)
    Wproj = np.asarray(Wproj, dtype=np.float32)
    bproj = np.asarray(bproj, dtype=np.float32)
    q_gamma = np.asarray(q_gamma, dtype=np.float32)
    q_beta = np.asarray(q_beta, dtype=np.float32)
    k_gamma = np.asarray(k_gamma, dtype=np.float32)
    k_beta = np.asarray(k_beta, dtype=np.float32)

    affine = not (np.all(q_gamma == 1.0) and np.all(q_beta == 0.0)
                  and np.all(k_gamma == 1.0) and np.all(k_beta == 0.0))
    nc = _get_nc(affine)

    bf = ml_dtypes.bfloat16
    Wqkv_b = np.ascontiguousarray(Wqkv.astype(bf))
    Wproj_b = np.ascontiguousarray(Wproj.astype(bf))
    beff = (bqkv[2 * E:].astype(np.float64) @ Wproj.astype(np.float64)
            + bproj.astype(np.float64)).astype(np.float32).astype(bf)

    in_maps = []
    for c in range(NCORES):
        b, half = divmod(c, 2)
        # kv token order = [own query half | other half]; attention output
        # is invariant to kv permutation and the q pass reads cols 0:TQ.
        xb = np.roll(x[b], -half * TQ, axis=0) if half else x[b]
        xT_kv = np.ascontiguousarray(xb.T.astype(bf))
        m = {
            "xT_kv": xT_kv,
            "Wqkv": Wqkv_b, "bqkv": bqkv, "beff": beff,
            "Wproj": Wproj_b,
        }
        if affine:
            m.update({"q_gamma": q_gamma, "q_beta": q_beta,
                      "k_gamma": k_gamma, "k_beta": k_beta})
        in_maps.append(m)

    global _last_in_maps
    _last_in_maps = in_maps
    res = run_bass_kernel_spmd(nc, in_maps, core_ids=list(range(NCORES)))
    y = np.empty((B, T, E), dtype=np.float32)
    for c in range(NCORES):
        b, half = divmod(c, 2)
        y[b, half * TQ:(half + 1) * TQ, :] = res.results[c]["out"]
    return y
